# revision 15
# baseline (speedup 1.0000x reference)
"""Multi-head self-attention (B=2, N=2048, C=1024, H=16) on 8 trn2 NeuronCores.

Sharding: core = b * 4 + g  (data parallel over batch B=2, tensor parallel
over 4 head-groups of 4 heads each).  Each core computes its head-group's
QKV projections, attention, and a partial output projection; the host sums
the 4 partials per batch (the "all-reduce") and adds the bias.

On-chip layout is fully "feature-on-partition" (transposed): the kernel
consumes x^T and produces out^T, so every matmul contracts along the
partition dim with no on-chip transposes.  Softmax runs along the key dim
which lives on partitions: the row-sum comes from augmenting V with 64
columns of ones (the PE computes sum(exp(S)) replicated across 64
partitions), and exp() needs no max-subtraction because scores are O(6).

All matmul operands are bf16 (the PE streams 2 bf16 moving elements per
cycle and ScalarE writes bf16 at 2x) with fp32 PSUM accumulation; the
softmax normalization (reciprocal and scaling) runs in fp32.  Head pairs
are interleaved so their K=64 score matmuls occupy different PE row-groups
and overlap in hardware.
"""

import sys

for _p in ("/opt/trn_rl_repo",):
    if _p not in sys.path:
        sys.path.append(_p)

import numpy as np

import concourse.bass as bass
import concourse.mybir as mybir
import concourse.tile as tile
from concourse import bacc
from concourse.bass_utils import run_bass_kernel_spmd

B, N, C = 2, 2048, 1024
H = 16
HS = C // H  # 64
G = 4  # head groups (tensor-parallel factor)
HPG = H // G  # heads per group = 4
GC = HPG * HS  # channels per group = 256
SCALE = HS**-0.5
P = 128
F32 = mybir.dt.float32
BF16 = mybir.dt.bfloat16

_CACHED = {}


def build_bass(loop_n=1, stage_in_loop=True, parts=("qkv2", "att2", "out"), msplit=False, s3=False, interleave_out=True, av_split=False, v_split=False, slack=False, v_first=False, no_exp=False, no_av=False, no_pair=False, no_scores=False, out_bf16=True, nstage=True, hoist_ones=True, norm2=True):
    nc = bacc.Bacc("TRN2", target_bir_lowering=False, debug=False)
    xT = nc.dram_tensor("xT", (C, N), BF16, kind="ExternalInput").ap()
    wqT = nc.dram_tensor("wqT", (C, GC), BF16, kind="ExternalInput").ap()
    wkT = nc.dram_tensor("wkT", (C, GC), BF16, kind="ExternalInput").ap()
    wvT = nc.dram_tensor("wvT", (C, GC), BF16, kind="ExternalInput").ap()
    woT = nc.dram_tensor("woT", (GC, C), BF16, kind="ExternalInput").ap()
    outT = nc.dram_tensor("outT", (C, N), BF16 if out_bf16 else F32, kind="ExternalOutput").ap()

    KC = C // P  # 8 contraction chunks for the qkv projection
    MC = N // P  # 16 sequence chunks
    QC = GC // P  # 2 chunks of group channels

    with tile.TileContext(nc) as tc:
        import contextlib

        ctx = contextlib.ExitStack()
        with ctx:
            wpool = ctx.enter_context(tc.tile_pool(name="wpool", bufs=1))
            mpool = ctx.enter_context(tc.tile_pool(name="mpool", bufs=1))
            psum = ctx.enter_context(tc.tile_pool(name="psum", bufs=1, space="PSUM"))
            opool = ctx.enter_context(tc.tile_pool(name="opool", bufs=4))

            # ---- persistent tiles ------------------------------------------
            xr = mpool.tile([P, KC, N], BF16)  # x^T
            wqr = wpool.tile([P, KC, GC], BF16)
            wkr = wpool.tile([P, KC, GC], BF16)
            wvr = wpool.tile([P, KC, GC], BF16)
            wor = wpool.tile([P, QC, C], BF16)
            qr = mpool.tile([P, QC, N], BF16)  # Q^T for the group
            kr = mpool.tile([P, QC, N], BF16)  # K^T
            # va blocks: even heads [V | ones], odd heads [ones | V] so the
            # attention output lands on the partition half matching the
            # head's slot in `an` (channels of chunk c = head 2c then 2c+1).
            va = mpool.tile([P, MC, HPG, P], BF16)
            ones_f = None if hoist_ones else mpool.tile([P, 2, HS], F32)
            an = mpool.tile([P, QC, N], BF16)  # normalized attn^T

            # ---- input loads (all bf16, direct DMA, 3 DGE queues) ----------
            def stage_all():
                x3 = xT.rearrange("(c p) n -> p c n", p=P)
                if nstage:
                    # n-sliced x loads ordered to match qk_proj's nch
                    # consumption; wk first so the first K-proj acc can
                    # start after just wk + x[n0].
                    wk3 = wkT.rearrange("(c p) m -> p c m", p=P)
                    nc.sync.dma_start(out=wkr[:], in_=wk3)
                    nc.scalar.dma_start(out=xr[:, :, 0:512], in_=x3[:, :, 0:512])
                    nc.gpsimd.dma_start(
                        out=wqr[:], in_=wqT.rearrange("(c p) m -> p c m", p=P)
                    )
                    nc.sync.dma_start(out=xr[:, :, 512:1024], in_=x3[:, :, 512:1024])
                    nc.scalar.dma_start(out=xr[:, :, 1024:1536], in_=x3[:, :, 1024:1536])
                    nc.gpsimd.dma_start(out=xr[:, :, 1536:2048], in_=x3[:, :, 1536:2048])
                    nc.sync.dma_start(
                        out=wvr[:], in_=wvT.rearrange("(c p) m -> p c m", p=P)
                    )
                    nc.gpsimd.dma_start(
                        out=wor[:], in_=woT.rearrange("(c p) o -> p c o", p=P)
                    )
                else:
                    engs = [nc.sync, nc.gpsimd, nc.scalar]
                    for j in range(4):
                        engs[j % 3].dma_start(
                            out=xr[:, 2 * j : 2 * j + 2, :], in_=x3[:, 2 * j : 2 * j + 2, :]
                        )
                    for i, (w_dram, w_r) in enumerate(
                        ((wqT, wqr), (wkT, wkr), (wvT, wvr))
                    ):
                        engs[(1 + i) % 3].dma_start(
                            out=w_r[:], in_=w_dram.rearrange("(c p) m -> p c m", p=P)
                        )
                    nc.gpsimd.dma_start(
                        out=wor[:], in_=woT.rearrange("(c p) o -> p c o", p=P)
                    )

            def packed_mm(acc, lhsT_full, rhs, start, stop):
                if msplit:
                    for hh in range(2):
                        nc.tensor.matmul(
                            acc[hh * 64 : (hh + 1) * 64, :],
                            lhsT_full[:, hh * 64 : (hh + 1) * 64],
                            rhs,
                            start=start,
                            stop=stop,
                        )
                else:
                    nc.tensor.matmul(acc[:], lhsT_full[:], rhs, start=start, stop=stop)

            # ---- phase B: QKV projections ----------------------------------
            def qk_proj(w_r, dst, mch):
                for nch in range(4):
                    acc = (
                        psum.tile([P, 1024], F32, tag="s", bufs=3, name="acc")[:, 0:512]
                        if s3
                        else psum.tile([P, 512], F32, tag="pb", bufs=2, name="acc")
                    )
                    for k in range(KC):
                        packed_mm(
                            acc,
                            w_r[:, k, mch * P : (mch + 1) * P],
                            xr[:, k, nch * 512 : (nch + 1) * 512],
                            k == 0,
                            k == KC - 1,
                        )
                    nc.vector.tensor_copy(dst[:, mch, nch * 512 : (nch + 1) * 512], acc[:])

            def v_proj():
                for m in range(MC):
                    vacc = (
                        psum.tile([P, 1024], F32, tag="s", bufs=3, name="vacc")[:, 0:GC]
                        if s3 else psum.tile([P, GC], F32, tag="pb", bufs=2, name="vacc")
                    )
                    for k in range(KC):
                        if v_split:
                            for hh in range(2):
                                nc.tensor.matmul(
                                    vacc[hh * 64 : (hh + 1) * 64, :],
                                    xr[:, k, m * P + hh * 64 : m * P + (hh + 1) * 64],
                                    wvr[:, k, :],
                                    start=(k == 0),
                                    stop=(k == KC - 1),
                                )
                        else:
                            packed_mm(
                                vacc,
                                xr[:, k, m * P : (m + 1) * P],
                                wvr[:, k, :],
                                k == 0,
                                k == KC - 1,
                            )
                    vh = vacc.rearrange("p (h e) -> p h e", h=HPG)
                    nc.vector.tensor_copy(va[:, m, 0::2, 0:HS], vh[:, 0::2, :])
                    nc.vector.tensor_copy(va[:, m, 1::2, HS:P], vh[:, 1::2, :])
                    if not hoist_ones:
                        nc.vector.tensor_copy(va[:, m, 0::2, HS:P], ones_f[:])
                        nc.vector.tensor_copy(va[:, m, 1::2, 0:HS], ones_f[:])

            # ---- phase C: attention for a head pair (2hp, 2hp+1) -----------
            # The two heads' K=64 score matmuls sit at base partitions 0 and
            # 64 -> distinct PE row-groups, so back-to-back emission lets the
            # hardware overlap them.  One exp covers both heads' P tiles.
            def attention_pair(hp, after_q=None):
                for q in range(4):  # query quarters of 512
                    qsl = slice(q * 512, (q + 1) * 512)
                    att0 = psum.tile([P, 512], F32, tag="att0", bufs=1, name="att0")
                    att1 = psum.tile([P, 512], F32, tag="att1", bufs=1, name="att1")
                    for m in range(MC):
                        s = psum.tile([P, 1024], F32, tag="s", bufs=3 if s3 else 2, name="s")
                        sw = 8 if no_scores else 512  # tiny-ablation width
                        for par, off in ((0, 0), (1, 64)):
                            o = 0 if no_pair else off
                            nc.tensor.matmul(
                                s[:, par * 512 : par * 512 + sw],
                                kr[o : o + 64, hp, m * P : (m + 1) * P],
                                qr[o : o + 64, hp, qsl.start : qsl.start + sw],
                                start=True,
                                stop=True,
                            )
                        p_sb = mpool.tile([P, 1024], BF16, tag="p_sb", bufs=8 if slack else 4, name="p_sb")
                        ew = 8 if no_exp else 1024
                        nc.scalar.activation(
                            p_sb[:, 0:ew], s[:, 0:ew], mybir.ActivationFunctionType.Exp, scale=SCALE
                        )
                        aw = 8 if no_av else 512
                        for par, att in ((0, att0), (1, att1)):
                            if av_split:
                                for hh in range(2):
                                    nc.tensor.matmul(
                                        att[hh * 64 : (hh + 1) * 64, :],
                                        va[:, m, 2 * hp + par, hh * 64 : (hh + 1) * 64],
                                        p_sb[:, par * 512 : (par + 1) * 512],
                                        start=(m == 0),
                                        stop=(m == MC - 1),
                                    )
                            else:
                                packed_mm(
                                    att[:, 0:aw] if aw != 512 else att,
                                    va[:, m, 2 * hp + par, :],
                                    p_sb[:, par * 512 : par * 512 + aw],
                                    m == 0,
                                    m == MC - 1,
                                )
                    # normalize.  Even head: attn rows 0:64, rowsum 64:128;
                    # odd head flipped (va block order).  The custom recip
                    # uop only works at base partition 0; cross-partition
                    # moves go through SBUF->SBUF DMA.
                    if norm2:
                        # read att PSUM directly in the muls; odd head's recip
                        # also reads PSUM directly (sums already at rows 0:64).
                        # rr DMAs ride the scalar HWDGE queue, away from outT.
                        au0 = mpool.tile([P, 512], F32, tag="au", bufs=4, name="au0")
                        rr0 = mpool.tile([P, 512], F32, tag="rr", bufs=4, name="rr0")
                        rr1 = mpool.tile([P, 512], F32, tag="rr", bufs=4, name="rr1")
                        nc.vector.tensor_copy(au0[64:128, :], att0[64:128, :])
                        nc.scalar.dma_start(out=rr0[0:64, :], in_=au0[64:128, :])
                        nc.vector.reciprocal_approx_fast(rr1[0:64, :], att1[0:64, :])
                        nc.scalar.dma_start(out=rr1[64:128, :], in_=rr1[0:64, :])
                        nc.vector.reciprocal_approx_fast(rr0[0:64, :], rr0[0:64, :])
                        nc.vector.tensor_mul(an[0:64, hp, qsl], att0[0:64, :], rr0[0:64, :])
                        nc.vector.tensor_mul(
                            an[64:128, hp, qsl], att1[64:128, :], rr1[64:128, :]
                        )
                    else:
                        au0 = mpool.tile([P, 512], F32, tag="au", bufs=6 if slack else 4, name="au0")
                        au1 = mpool.tile([P, 512], F32, tag="au", bufs=6 if slack else 4, name="au1")
                        rr0 = mpool.tile([P, 512], F32, tag="rr", bufs=6 if slack else 4, name="rr0")
                        rr1 = mpool.tile([P, 512], F32, tag="rr", bufs=6 if slack else 4, name="rr1")
                        nc.vector.tensor_copy(au0[:], att0[:])
                        nc.vector.tensor_copy(au1[:], att1[:])
                        (nc.gpsimd if slack else nc.sync).dma_start(out=rr0[0:64, :], in_=au0[64:128, :])
                        nc.vector.reciprocal_approx_fast(rr0[0:64, :], rr0[0:64, :])
                        nc.vector.tensor_mul(an[0:64, hp, qsl], au0[0:64, :], rr0[0:64, :])
                        nc.vector.reciprocal_approx_fast(rr1[0:64, :], au1[0:64, :])
                        (nc.gpsimd if slack else nc.sync).dma_start(out=rr1[64:128, :], in_=rr1[0:64, :])
                        nc.vector.tensor_mul(
                            an[64:128, hp, qsl], au1[64:128, :], rr1[64:128, :]
                        )
                    if after_q is not None:
                        after_q(q)

            # ---- phase E: output projection (one query quarter) ------------
            def out_proj_quarter(nch):
                for och in range(C // P):
                    o_ps = (
                        psum.tile([P, 1024], F32, tag="s", bufs=3, name="o_ps")[:, 0:512]
                        if s3
                        else psum.tile([P, 512], F32, tag="pb", bufs=2, name="o_ps")
                    )
                    for c in range(QC):
                        packed_mm(
                            o_ps,
                            wor[:, c, och * P : (och + 1) * P],
                            an[:, c, nch * 512 : (nch + 1) * 512],
                            c == 0,
                            c == QC - 1,
                        )
                    o_sb = opool.tile([P, 512], BF16 if out_bf16 else F32, name="o_sb")
                    nc.vector.tensor_copy(o_sb[:], o_ps[:])
                    eng = nc.sync if och % 2 == 0 else nc.gpsimd
                    eng.dma_start(
                        out=outT[och * P : (och + 1) * P, nch * 512 : (nch + 1) * 512],
                        in_=o_sb,
                    )

            # ---- body: emission order enables PE/ACT overlap ---------------
            def body(staged):
                if not hoist_ones:
                    nc.vector.memset(ones_f, 1.0)
                if staged:
                    stage_all()
                if v_first:
                    qk_proj(wkr, kr, 0)
                    v_proj()
                    qk_proj(wqr, qr, 0)
                else:
                    qk_proj(wkr, kr, 0)
                    qk_proj(wqr, qr, 0)
                    v_proj()
                attention_pair(0)
                if "qkv2" in parts:
                    qk_proj(wkr, kr, 1)
                    qk_proj(wqr, qr, 1)
                after = out_proj_quarter if ("out" in parts and interleave_out) else None
                if "att2" in parts:
                    attention_pair(1, after_q=after)
                if "out" in parts and after is None:
                    for q in range(4):
                        out_proj_quarter(q)

            if hoist_ones:
                # ones columns of va never change; fill them once up front
                nc.vector.memset(va[:, :, 0::2, HS:P], 1.0)
                nc.vector.memset(va[:, :, 1::2, 0:HS], 1.0)
            if loop_n > 1:
                if not stage_in_loop:
                    stage_all()
                ET = mybir.EngineType
                with tc.For_i(
                    0,
                    loop_n,
                    1,
                    hint_engines=(ET.PE, ET.Activation, ET.DVE, ET.SP),
                ):
                    body(staged=stage_in_loop)
            elif loop_n < 0:  # python-unrolled -loop_n bodies (sim analysis)
                for _ in range(-loop_n):
                    body(staged=True)
            else:
                body(staged=True)

    nc.compile()
    return nc


def shard_inputs(x, w_qkv, w_out):
    """Host-side shard prep. Returns in_maps for cores 0..7 (core = b*4+g).

    All inputs ship as bf16 (the PE consumes bf16 directly at 2x moving
    rate); accumulation on chip is fp32 and the output returns fp32."""
    import ml_dtypes

    bf16 = ml_dtypes.bfloat16
    # w_qkv row d = c_idx*3 + t  (t: 0=q, 1=k, 2=v)  [stride-3 interleave]
    wr = np.ascontiguousarray(w_qkv.reshape(C, 3, C))
    in_maps = []
    for b in range(B):
        xTb = np.ascontiguousarray(x[b].T.astype(bf16))
        for g in range(G):
            sl = slice(g * GC, (g + 1) * GC)
            in_maps.append(
                {
                    "xT": xTb,
                    "wqT": np.ascontiguousarray(wr[sl, 0, :].T.astype(bf16)),
                    "wkT": np.ascontiguousarray(wr[sl, 1, :].T.astype(bf16)),
                    "wvT": np.ascontiguousarray(wr[sl, 2, :].T.astype(bf16)),
                    "woT": np.ascontiguousarray(w_out[:, sl].T.astype(bf16)),
                }
            )
    return in_maps


def kernel(x, w_qkv, w_out, b_out):
    x = np.asarray(x, dtype=np.float32)
    w_qkv = np.asarray(w_qkv, dtype=np.float32)
    w_out = np.asarray(w_out, dtype=np.float32)
    b_out = np.asarray(b_out, dtype=np.float32)

    if "nc" not in _CACHED:
        _CACHED["nc"] = build_bass()
    nc = _CACHED["nc"]

    in_maps = shard_inputs(x, w_qkv, w_out)
    res = run_bass_kernel_spmd(nc, in_maps, core_ids=list(range(8)))

    out = np.empty((B, N, C), dtype=np.float32)
    for b in range(B):
        acc = res.results[b * G + 0]["outT"].astype(np.float32)
        for g in range(1, G):
            acc = acc + res.results[b * G + g]["outT"].astype(np.float32)
        out[b] = acc.T + b_out
    return out


if __name__ == "__main__":
    rng = np.random.default_rng(0)
    x = rng.standard_normal((B, N, C), dtype=np.float32)
    w_qkv = rng.standard_normal((3 * C, C), dtype=np.float32) * C**-0.5
    w_out = rng.standard_normal((C, C), dtype=np.float32) * C**-0.5
    b_out = np.zeros((C,), dtype=np.float32)
    got = kernel(x, w_qkv, w_out, b_out)
    print("kernel ran, output shape", got.shape)



# revision 23
# speedup vs baseline: 1.0196x; 1.0196x over previous
"""Multi-head self-attention (B=2, N=2048, C=1024, H=16) on 8 trn2 NeuronCores.

Sharding: core = b * 4 + g  (data parallel over batch B=2, tensor parallel
over 4 head-groups of 4 heads each).  Each core computes its head-group's
QKV projections, attention, and a partial output projection; the host sums
the 4 partials per batch (the "all-reduce") and adds the bias.

On-chip layout is fully "feature-on-partition" (transposed): the kernel
consumes x^T and produces out^T, so every matmul contracts along the
partition dim with no on-chip transposes.  Softmax runs along the key dim
which lives on partitions: the row-sum comes from augmenting V with 64
columns of ones (the PE computes sum(exp(S)) replicated across 64
partitions), and exp() needs no max-subtraction because scores are O(6).

All matmul operands are bf16 (the PE streams 2 bf16 moving elements per
cycle and ScalarE writes bf16 at 2x) with fp32 PSUM accumulation; the
softmax normalization (reciprocal and scaling) runs in fp32.  Head pairs
are interleaved so their K=64 score matmuls occupy different PE row-groups
and overlap in hardware.
"""

import sys

for _p in ("/opt/trn_rl_repo",):
    if _p not in sys.path:
        sys.path.append(_p)

import numpy as np

import concourse.bass as bass
import concourse.mybir as mybir
import concourse.tile as tile
from concourse import bacc
from concourse.bass_utils import run_bass_kernel_spmd

B, N, C = 2, 2048, 1024
H = 16
HS = C // H  # 64
G = 4  # head groups (tensor-parallel factor)
HPG = H // G  # heads per group = 4
GC = HPG * HS  # channels per group = 256
SCALE = HS**-0.5
P = 128
F32 = mybir.dt.float32
BF16 = mybir.dt.bfloat16

_CACHED = {}


def build_bass(loop_n=1, stage_in_loop=True, parts=("qkv2", "att2", "out"), msplit=False, s3=False, interleave_out=False, av_split=False, v_split=False, slack=False, v_first=False, no_exp=False, no_av=False, no_pair=False, no_scores=False, no_qkv=False, no_out=False, no_ldw=False, out_bf16=False, nstage=False, hoist_ones=False, norm2=False):
    nc = bacc.Bacc("TRN2", target_bir_lowering=False, debug=False)
    xT = nc.dram_tensor("xT", (C, N), BF16, kind="ExternalInput").ap()
    wqT = nc.dram_tensor("wqT", (C, GC), BF16, kind="ExternalInput").ap()
    wkT = nc.dram_tensor("wkT", (C, GC), BF16, kind="ExternalInput").ap()
    wvT = nc.dram_tensor("wvT", (C, GC), BF16, kind="ExternalInput").ap()
    woT = nc.dram_tensor("woT", (GC, C), BF16, kind="ExternalInput").ap()
    outT = nc.dram_tensor("outT", (C, N), BF16 if out_bf16 else F32, kind="ExternalOutput").ap()

    KC = C // P  # 8 contraction chunks for the qkv projection
    MC = N // P  # 16 sequence chunks
    QC = GC // P  # 2 chunks of group channels

    with tile.TileContext(nc) as tc:
        import contextlib

        ctx = contextlib.ExitStack()
        with ctx:
            wpool = ctx.enter_context(tc.tile_pool(name="wpool", bufs=1))
            mpool = ctx.enter_context(tc.tile_pool(name="mpool", bufs=1))
            psum = ctx.enter_context(tc.tile_pool(name="psum", bufs=1, space="PSUM"))
            opool = ctx.enter_context(tc.tile_pool(name="opool", bufs=4))

            # ---- persistent tiles ------------------------------------------
            xr = mpool.tile([P, KC, N], BF16)  # x^T
            wqr = wpool.tile([P, KC, GC], BF16)
            wkr = wpool.tile([P, KC, GC], BF16)
            wvr = wpool.tile([P, KC, GC], BF16)
            wor = wpool.tile([P, QC, C], BF16)
            qr = mpool.tile([P, QC, N], BF16)  # Q^T for the group
            kr = mpool.tile([P, QC, N], BF16)  # K^T
            # va blocks: even heads [V | ones], odd heads [ones | V] so the
            # attention output lands on the partition half matching the
            # head's slot in `an` (channels of chunk c = head 2c then 2c+1).
            va = mpool.tile([P, MC, HPG, P], BF16)
            ones_f = None if hoist_ones else mpool.tile([P, 2, HS], F32)
            an = mpool.tile([P, QC, N], BF16)  # normalized attn^T

            # ---- input loads (all bf16, direct DMA, 3 DGE queues) ----------
            def stage_all():
                x3 = xT.rearrange("(c p) n -> p c n", p=P)
                if nstage:
                    # n-sliced x loads ordered to match qk_proj's nch
                    # consumption; wk first so the first K-proj acc can
                    # start after just wk + x[n0].
                    wk3 = wkT.rearrange("(c p) m -> p c m", p=P)
                    nc.sync.dma_start(out=wkr[:], in_=wk3)
                    nc.scalar.dma_start(out=xr[:, :, 0:512], in_=x3[:, :, 0:512])
                    nc.gpsimd.dma_start(
                        out=wqr[:], in_=wqT.rearrange("(c p) m -> p c m", p=P)
                    )
                    nc.sync.dma_start(out=xr[:, :, 512:1024], in_=x3[:, :, 512:1024])
                    nc.scalar.dma_start(out=xr[:, :, 1024:1536], in_=x3[:, :, 1024:1536])
                    nc.gpsimd.dma_start(out=xr[:, :, 1536:2048], in_=x3[:, :, 1536:2048])
                    nc.sync.dma_start(
                        out=wvr[:], in_=wvT.rearrange("(c p) m -> p c m", p=P)
                    )
                    nc.gpsimd.dma_start(
                        out=wor[:], in_=woT.rearrange("(c p) o -> p c o", p=P)
                    )
                else:
                    engs = [nc.sync, nc.gpsimd, nc.scalar]
                    for j in range(4):
                        engs[j % 3].dma_start(
                            out=xr[:, 2 * j : 2 * j + 2, :], in_=x3[:, 2 * j : 2 * j + 2, :]
                        )
                    for i, (w_dram, w_r) in enumerate(
                        ((wqT, wqr), (wkT, wkr), (wvT, wvr))
                    ):
                        engs[(1 + i) % 3].dma_start(
                            out=w_r[:], in_=w_dram.rearrange("(c p) m -> p c m", p=P)
                        )
                    nc.gpsimd.dma_start(
                        out=wor[:], in_=woT.rearrange("(c p) o -> p c o", p=P)
                    )

            def packed_mm(acc, lhsT_full, rhs, start, stop):
                if msplit:
                    for hh in range(2):
                        nc.tensor.matmul(
                            acc[hh * 64 : (hh + 1) * 64, :],
                            lhsT_full[:, hh * 64 : (hh + 1) * 64],
                            rhs,
                            start=start,
                            stop=stop,
                        )
                else:
                    nc.tensor.matmul(acc[:], lhsT_full[:], rhs, start=start, stop=stop)

            # ---- phase B: QKV projections ----------------------------------
            def qk_proj(w_r, dst, mch):
                for nch in range(4):
                    acc = (
                        psum.tile([P, 1024], F32, tag="s", bufs=3, name="acc")[:, 0:512]
                        if s3
                        else psum.tile([P, 512], F32, tag="pb", bufs=2, name="acc")
                    )
                    qw = 8 if no_qkv else 512
                    lw = 8 if no_ldw else P
                    for k in range(KC):
                        packed_mm(
                            acc[0:lw, 0:qw],
                            w_r[:, k, mch * P : mch * P + lw],
                            xr[:, k, nch * 512 : nch * 512 + qw],
                            k == 0,
                            k == KC - 1,
                        )
                    if mch == 0:  # ACT is idle in phase 1; DVE during att0
                        nc.scalar.copy(dst[:, mch, nch * 512 : (nch + 1) * 512], acc[:])
                    else:
                        nc.vector.tensor_copy(dst[:, mch, nch * 512 : (nch + 1) * 512], acc[:])

            def v_proj():
                for m in range(MC):
                    vacc = (
                        psum.tile([P, 1024], F32, tag="s", bufs=3, name="vacc")[:, 0:GC]
                        if s3 else psum.tile([P, GC], F32, tag="pb", bufs=2, name="vacc")
                    )
                    for k in range(KC):
                        if v_split:
                            for hh in range(2):
                                nc.tensor.matmul(
                                    vacc[hh * 64 : (hh + 1) * 64, :],
                                    xr[:, k, m * P + hh * 64 : m * P + (hh + 1) * 64],
                                    wvr[:, k, :],
                                    start=(k == 0),
                                    stop=(k == KC - 1),
                                )
                        else:
                            vw = 8 if no_qkv else GC
                            lw = 8 if no_ldw else P
                            packed_mm(
                                vacc[0:lw, 0:vw],
                                xr[:, k, m * P : m * P + lw],
                                wvr[:, k, 0:vw],
                                k == 0,
                                k == KC - 1,
                            )
                    vh = vacc.rearrange("p (h e) -> p h e", h=HPG)
                    nc.vector.tensor_copy(va[:, m, 0::2, 0:HS], vh[:, 0::2, :])
                    nc.vector.tensor_copy(va[:, m, 1::2, HS:P], vh[:, 1::2, :])
                    if not hoist_ones:
                        nc.vector.tensor_copy(va[:, m, 0::2, HS:P], ones_f[:])
                        nc.vector.tensor_copy(va[:, m, 1::2, 0:HS], ones_f[:])

            # ---- phase C: attention for a head pair (2hp, 2hp+1) -----------
            # The two heads' K=64 score matmuls sit at base partitions 0 and
            # 64 -> distinct PE row-groups, so back-to-back emission lets the
            # hardware overlap them.  One exp covers both heads' P tiles.
            def attention_pair(hp, after_q=None):
                for q in range(4):  # query quarters of 512
                    qsl = slice(q * 512, (q + 1) * 512)
                    att0 = psum.tile([P, 512], F32, tag="att0", bufs=1, name="att0")
                    att1 = psum.tile([P, 512], F32, tag="att1", bufs=1, name="att1")
                    for m in range(MC):
                        s = psum.tile([P, 1024], F32, tag="s", bufs=3 if s3 else 2, name="s")
                        sw = 8 if no_scores else 512  # tiny-ablation width
                        lw = 8 if no_ldw else P
                        for par, off in ((0, 0), (1, 64)):
                            o = 0 if no_pair else off
                            nc.tensor.matmul(
                                s[0:lw, par * 512 : par * 512 + sw],
                                kr[o : o + 64, hp, m * P : m * P + lw],
                                qr[o : o + 64, hp, qsl.start : qsl.start + sw],
                                start=True,
                                stop=True,
                            )
                        p_sb = mpool.tile([P, 1024], BF16, tag="p_sb", bufs=8 if slack else 4, name="p_sb")
                        ew = 8 if no_exp else 1024
                        nc.scalar.activation(
                            p_sb[:, 0:ew], s[:, 0:ew], mybir.ActivationFunctionType.Exp, scale=SCALE
                        )
                        aw = 8 if no_av else 512
                        for par, att in ((0, att0), (1, att1)):
                            if av_split:
                                for hh in range(2):
                                    nc.tensor.matmul(
                                        att[hh * 64 : (hh + 1) * 64, :],
                                        va[:, m, 2 * hp + par, hh * 64 : (hh + 1) * 64],
                                        p_sb[:, par * 512 : (par + 1) * 512],
                                        start=(m == 0),
                                        stop=(m == MC - 1),
                                    )
                            else:
                                packed_mm(
                                    att[0 : (8 if no_ldw else P), 0:aw],
                                    va[:, m, 2 * hp + par, 0 : (8 if no_ldw else P)],
                                    p_sb[:, par * 512 : par * 512 + aw],
                                    m == 0,
                                    m == MC - 1,
                                )
                    # normalize.  Even head: attn rows 0:64, rowsum 64:128;
                    # odd head flipped (va block order).  The custom recip
                    # uop only works at base partition 0; cross-partition
                    # moves go through SBUF->SBUF DMA.
                    if norm2:
                        # read att PSUM directly in the muls; odd head's recip
                        # also reads PSUM directly (sums already at rows 0:64).
                        # rr DMAs ride the scalar HWDGE queue, away from outT.
                        au0 = mpool.tile([P, 512], F32, tag="au", bufs=4, name="au0")
                        rr0 = mpool.tile([P, 512], F32, tag="rr", bufs=4, name="rr0")
                        rr1 = mpool.tile([P, 512], F32, tag="rr", bufs=4, name="rr1")
                        nc.vector.tensor_copy(au0[64:128, :], att0[64:128, :])
                        nc.scalar.dma_start(out=rr0[0:64, :], in_=au0[64:128, :])
                        nc.vector.reciprocal_approx_fast(rr1[0:64, :], att1[0:64, :])
                        nc.scalar.dma_start(out=rr1[64:128, :], in_=rr1[0:64, :])
                        nc.vector.reciprocal_approx_fast(rr0[0:64, :], rr0[0:64, :])
                        nc.vector.tensor_mul(an[0:64, hp, qsl], att0[0:64, :], rr0[0:64, :])
                        nc.vector.tensor_mul(
                            an[64:128, hp, qsl], att1[64:128, :], rr1[64:128, :]
                        )
                    else:
                        au0 = mpool.tile([P, 512], F32, tag="au", bufs=6 if slack else 4, name="au0")
                        au1 = mpool.tile([P, 512], F32, tag="au", bufs=6 if slack else 4, name="au1")
                        rr0 = mpool.tile([P, 512], F32, tag="rr", bufs=6 if slack else 4, name="rr0")
                        rr1 = mpool.tile([P, 512], F32, tag="rr", bufs=6 if slack else 4, name="rr1")
                        nc.vector.tensor_copy(au0[:], att0[:])
                        nc.vector.tensor_copy(au1[:], att1[:])
                        (nc.gpsimd if slack else nc.sync).dma_start(out=rr0[0:64, :], in_=au0[64:128, :])
                        nc.vector.reciprocal_approx_fast(rr0[0:64, :], rr0[0:64, :])
                        nc.vector.tensor_mul(an[0:64, hp, qsl], au0[0:64, :], rr0[0:64, :])
                        nc.vector.reciprocal_approx_fast(rr1[0:64, :], au1[0:64, :])
                        (nc.gpsimd if slack else nc.sync).dma_start(out=rr1[64:128, :], in_=rr1[0:64, :])
                        nc.vector.tensor_mul(
                            an[64:128, hp, qsl], au1[64:128, :], rr1[64:128, :]
                        )
                    if after_q is not None:
                        after_q(q)

            # ---- phase E: output projection (one query quarter) ------------
            # 8 och chunks collect into one SBUF tile; a single DMA ships the
            # whole [C, 512] quarter (1 issue instead of 8).
            def out_proj_quarter(nch):
                o_full = opool.tile(
                    [P, C // P, 512], BF16 if out_bf16 else F32, tag="o_full",
                    bufs=2, name="o_full",
                )
                for och in range(C // P):
                    o_ps = (
                        psum.tile([P, 1024], F32, tag="s", bufs=3, name="o_ps")[:, 0:512]
                        if s3
                        else psum.tile([P, 512], F32, tag="pb", bufs=2, name="o_ps")
                    )
                    ow = 8 if no_out else 512
                    lw = 8 if no_ldw else P
                    for c in range(QC):
                        packed_mm(
                            o_ps[0:lw, 0:ow],
                            wor[:, c, och * P : och * P + lw],
                            an[:, c, nch * 512 : nch * 512 + ow],
                            c == 0,
                            c == QC - 1,
                        )
                    nc.vector.tensor_copy(o_full[:, och, :], o_ps[:])
                eng = nc.sync if nch % 2 == 0 else nc.gpsimd
                eng.dma_start(
                    out=outT[:, nch * 512 : (nch + 1) * 512].rearrange(
                        "(o p) n -> p o n", p=P
                    ),
                    in_=o_full[:],
                )

            # ---- body: emission order enables PE/ACT overlap ---------------
            def body(staged):
                if not hoist_ones:
                    nc.vector.memset(ones_f, 1.0)
                if staged:
                    stage_all()
                if v_first:
                    qk_proj(wkr, kr, 0)
                    v_proj()
                    qk_proj(wqr, qr, 0)
                else:
                    qk_proj(wkr, kr, 0)
                    qk_proj(wqr, qr, 0)
                    v_proj()
                attention_pair(0)
                if "qkv2" in parts:
                    qk_proj(wkr, kr, 1)
                    qk_proj(wqr, qr, 1)
                after = out_proj_quarter if ("out" in parts and interleave_out) else None
                if "att2" in parts:
                    attention_pair(1, after_q=after)
                if "out" in parts and after is None:
                    for q in range(4):
                        out_proj_quarter(q)

            if hoist_ones:
                # ones columns of va never change; fill them once up front
                nc.vector.memset(va[:, :, 0::2, HS:P], 1.0)
                nc.vector.memset(va[:, :, 1::2, 0:HS], 1.0)
            if loop_n > 1:
                if not stage_in_loop:
                    stage_all()
                ET = mybir.EngineType
                with tc.For_i(
                    0,
                    loop_n,
                    1,
                    hint_engines=(ET.PE, ET.Activation, ET.DVE, ET.SP),
                ):
                    body(staged=stage_in_loop)
            elif loop_n < 0:  # python-unrolled -loop_n bodies (sim analysis)
                for _ in range(-loop_n):
                    body(staged=True)
            else:
                body(staged=True)

    nc.compile()
    return nc


def shard_inputs(x, w_qkv, w_out):
    """Host-side shard prep. Returns in_maps for cores 0..7 (core = b*4+g).

    All inputs ship as bf16 (the PE consumes bf16 directly at 2x moving
    rate); accumulation on chip is fp32 and the output returns fp32."""
    import ml_dtypes

    bf16 = ml_dtypes.bfloat16
    # w_qkv row d = c_idx*3 + t  (t: 0=q, 1=k, 2=v)  [stride-3 interleave]
    wr = np.ascontiguousarray(w_qkv.reshape(C, 3, C))
    in_maps = []
    for b in range(B):
        xTb = np.ascontiguousarray(x[b].T.astype(bf16))
        for g in range(G):
            sl = slice(g * GC, (g + 1) * GC)
            in_maps.append(
                {
                    "xT": xTb,
                    "wqT": np.ascontiguousarray(wr[sl, 0, :].T.astype(bf16)),
                    "wkT": np.ascontiguousarray(wr[sl, 1, :].T.astype(bf16)),
                    "wvT": np.ascontiguousarray(wr[sl, 2, :].T.astype(bf16)),
                    "woT": np.ascontiguousarray(w_out[:, sl].T.astype(bf16)),
                }
            )
    return in_maps


def kernel(x, w_qkv, w_out, b_out):
    x = np.asarray(x, dtype=np.float32)
    w_qkv = np.asarray(w_qkv, dtype=np.float32)
    w_out = np.asarray(w_out, dtype=np.float32)
    b_out = np.asarray(b_out, dtype=np.float32)

    if "nc" not in _CACHED:
        _CACHED["nc"] = build_bass()
    nc = _CACHED["nc"]

    in_maps = shard_inputs(x, w_qkv, w_out)
    res = run_bass_kernel_spmd(nc, in_maps, core_ids=list(range(8)))

    out = np.empty((B, N, C), dtype=np.float32)
    for b in range(B):
        acc = res.results[b * G + 0]["outT"].astype(np.float32)
        for g in range(1, G):
            acc = acc + res.results[b * G + g]["outT"].astype(np.float32)
        out[b] = acc.T + b_out
    return out


if __name__ == "__main__":
    rng = np.random.default_rng(0)
    x = rng.standard_normal((B, N, C), dtype=np.float32)
    w_qkv = rng.standard_normal((3 * C, C), dtype=np.float32) * C**-0.5
    w_out = rng.standard_normal((C, C), dtype=np.float32) * C**-0.5
    b_out = np.zeros((C,), dtype=np.float32)
    got = kernel(x, w_qkv, w_out, b_out)
    print("kernel ran, output shape", got.shape)



# revision 24
# speedup vs baseline: 1.1634x; 1.1411x over previous
"""Multi-head self-attention (B=2, N=2048, C=1024, H=16) on 8 trn2 NeuronCores.

Sharding: core = b * 4 + g  (data parallel over batch B=2, tensor parallel
over 4 head-groups of 4 heads each).  Each core computes its head-group's
QKV projections, attention, and a partial output projection; the host sums
the 4 partials per batch (the "all-reduce") and adds the bias.

On-chip layout is fully "feature-on-partition" (transposed): the kernel
consumes x^T and produces out^T, so every matmul contracts along the
partition dim with no on-chip transposes.  Softmax runs along the key dim
which lives on partitions: the row-sum comes from augmenting V with 64
columns of ones (the PE computes sum(exp(S)) replicated across 64
partitions), and exp() needs no max-subtraction because scores are O(6).

All matmul operands are bf16 (the PE streams 2 bf16 moving elements per
cycle and ScalarE writes bf16 at 2x) with fp32 PSUM accumulation; the
softmax normalization (reciprocal and scaling) runs in fp32.  Head pairs
are interleaved so their K=64 score matmuls occupy different PE row-groups
and overlap in hardware.

Optimization notes (measured on HW via R=65/193 differential timing;
run-to-run noise is +-15-20us, so only robust effects are listed):
- Engine budget per core-iteration: PE ~137-164us of matmul rows (scores
  pair-overlapped), ACT ~128us of exp (16.8M elems at 1 elem/lane/cycle,
  an algorithmic floor for this decomposition), DVE ~74us.
- Tiny-ablations (shrinking each instruction class to 8 columns while
  keeping all instructions/deps) show a ~137us "infrastructure floor"
  (DVE copies + DMA issues + semaphores + sequencer dispatch) that
  dominates; LDWEIGHTS serialization was ruled out (no_ldw ablation).
- fp8/DoubleRow is numerically dead here: e4m3 on any matmul pair
  (scores/AV/QKV/out) exceeds the 2e-2 rel-absmax budget (5e-2/2.2e-2/
  7.5e-2/3.4e-2 measured via CPU emulation).
- One matmul output <= 1 PSUM bank (512 fp32) blocks wider moving
  operands; PSUM (8 banks) is fully allocated: s(2x2) + att(2) + pb(2).
- Knobs interleave_out/s3/slack/nstage/out_bf16/norm2/hoist_ones were
  all measured neutral-to-worse head-to-head; defaults keep them off.
- outT ships as one DMA per 512-query quarter (8 och chunks batched)
  and phase-1 projection copies run on the otherwise-idle ScalarE.
"""

import sys

for _p in ("/opt/trn_rl_repo",):
    if _p not in sys.path:
        sys.path.append(_p)

import numpy as np

import concourse.bass as bass
import concourse.mybir as mybir
import concourse.tile as tile
from concourse import bacc
from concourse.bass_utils import run_bass_kernel_spmd

B, N, C = 2, 2048, 1024
H = 16
HS = C // H  # 64
G = 4  # head groups (tensor-parallel factor)
HPG = H // G  # heads per group = 4
GC = HPG * HS  # channels per group = 256
SCALE = HS**-0.5
P = 128
F32 = mybir.dt.float32
BF16 = mybir.dt.bfloat16

_CACHED = {}


def build_bass(loop_n=1, stage_in_loop=True, parts=("qkv2", "att2", "out"), msplit=False, s3=False, interleave_out=False, av_split=False, v_split=False, slack=False, v_first=False, no_exp=False, no_av=False, no_pair=False, no_scores=False, no_qkv=False, no_out=False, no_ldw=False, out_bf16=False, nstage=False, hoist_ones=False, norm2=False):
    nc = bacc.Bacc("TRN2", target_bir_lowering=False, debug=False)
    xT = nc.dram_tensor("xT", (C, N), BF16, kind="ExternalInput").ap()
    wqT = nc.dram_tensor("wqT", (C, GC), BF16, kind="ExternalInput").ap()
    wkT = nc.dram_tensor("wkT", (C, GC), BF16, kind="ExternalInput").ap()
    wvT = nc.dram_tensor("wvT", (C, GC), BF16, kind="ExternalInput").ap()
    woT = nc.dram_tensor("woT", (GC, C), BF16, kind="ExternalInput").ap()
    outT = nc.dram_tensor("outT", (C, N), BF16 if out_bf16 else F32, kind="ExternalOutput").ap()

    KC = C // P  # 8 contraction chunks for the qkv projection
    MC = N // P  # 16 sequence chunks
    QC = GC // P  # 2 chunks of group channels

    with tile.TileContext(nc) as tc:
        import contextlib

        ctx = contextlib.ExitStack()
        with ctx:
            wpool = ctx.enter_context(tc.tile_pool(name="wpool", bufs=1))
            mpool = ctx.enter_context(tc.tile_pool(name="mpool", bufs=1))
            psum = ctx.enter_context(tc.tile_pool(name="psum", bufs=1, space="PSUM"))
            opool = ctx.enter_context(tc.tile_pool(name="opool", bufs=4))

            # ---- persistent tiles ------------------------------------------
            xr = mpool.tile([P, KC, N], BF16)  # x^T
            wqr = wpool.tile([P, KC, GC], BF16)
            wkr = wpool.tile([P, KC, GC], BF16)
            wvr = wpool.tile([P, KC, GC], BF16)
            wor = wpool.tile([P, QC, C], BF16)
            qr = mpool.tile([P, QC, N], BF16)  # Q^T for the group
            kr = mpool.tile([P, QC, N], BF16)  # K^T
            # va blocks: even heads [V | ones], odd heads [ones | V] so the
            # attention output lands on the partition half matching the
            # head's slot in `an` (channels of chunk c = head 2c then 2c+1).
            va = mpool.tile([P, MC, HPG, P], BF16)
            ones_f = None if hoist_ones else mpool.tile([P, 2, HS], F32)
            an = mpool.tile([P, QC, N], BF16)  # normalized attn^T

            # ---- input loads (all bf16, direct DMA, 3 DGE queues) ----------
            def stage_all():
                x3 = xT.rearrange("(c p) n -> p c n", p=P)
                if nstage:
                    # n-sliced x loads ordered to match qk_proj's nch
                    # consumption; wk first so the first K-proj acc can
                    # start after just wk + x[n0].
                    wk3 = wkT.rearrange("(c p) m -> p c m", p=P)
                    nc.sync.dma_start(out=wkr[:], in_=wk3)
                    nc.scalar.dma_start(out=xr[:, :, 0:512], in_=x3[:, :, 0:512])
                    nc.gpsimd.dma_start(
                        out=wqr[:], in_=wqT.rearrange("(c p) m -> p c m", p=P)
                    )
                    nc.sync.dma_start(out=xr[:, :, 512:1024], in_=x3[:, :, 512:1024])
                    nc.scalar.dma_start(out=xr[:, :, 1024:1536], in_=x3[:, :, 1024:1536])
                    nc.gpsimd.dma_start(out=xr[:, :, 1536:2048], in_=x3[:, :, 1536:2048])
                    nc.sync.dma_start(
                        out=wvr[:], in_=wvT.rearrange("(c p) m -> p c m", p=P)
                    )
                    nc.gpsimd.dma_start(
                        out=wor[:], in_=woT.rearrange("(c p) o -> p c o", p=P)
                    )
                else:
                    engs = [nc.sync, nc.gpsimd, nc.scalar]
                    for j in range(4):
                        engs[j % 3].dma_start(
                            out=xr[:, 2 * j : 2 * j + 2, :], in_=x3[:, 2 * j : 2 * j + 2, :]
                        )
                    for i, (w_dram, w_r) in enumerate(
                        ((wqT, wqr), (wkT, wkr), (wvT, wvr))
                    ):
                        engs[(1 + i) % 3].dma_start(
                            out=w_r[:], in_=w_dram.rearrange("(c p) m -> p c m", p=P)
                        )
                    nc.gpsimd.dma_start(
                        out=wor[:], in_=woT.rearrange("(c p) o -> p c o", p=P)
                    )

            def packed_mm(acc, lhsT_full, rhs, start, stop):
                if msplit:
                    for hh in range(2):
                        nc.tensor.matmul(
                            acc[hh * 64 : (hh + 1) * 64, :],
                            lhsT_full[:, hh * 64 : (hh + 1) * 64],
                            rhs,
                            start=start,
                            stop=stop,
                        )
                else:
                    nc.tensor.matmul(acc[:], lhsT_full[:], rhs, start=start, stop=stop)

            # ---- phase B: QKV projections ----------------------------------
            def qk_proj(w_r, dst, mch):
                for nch in range(4):
                    acc = (
                        psum.tile([P, 1024], F32, tag="s", bufs=3, name="acc")[:, 0:512]
                        if s3
                        else psum.tile([P, 512], F32, tag="pb", bufs=2, name="acc")
                    )
                    qw = 8 if no_qkv else 512
                    lw = 8 if no_ldw else P
                    for k in range(KC):
                        packed_mm(
                            acc[0:lw, 0:qw],
                            w_r[:, k, mch * P : mch * P + lw],
                            xr[:, k, nch * 512 : nch * 512 + qw],
                            k == 0,
                            k == KC - 1,
                        )
                    if mch == 0:  # ACT is idle in phase 1; DVE during att0
                        nc.scalar.copy(dst[:, mch, nch * 512 : (nch + 1) * 512], acc[:])
                    else:
                        nc.vector.tensor_copy(dst[:, mch, nch * 512 : (nch + 1) * 512], acc[:])

            def v_proj():
                for m in range(MC):
                    vacc = (
                        psum.tile([P, 1024], F32, tag="s", bufs=3, name="vacc")[:, 0:GC]
                        if s3 else psum.tile([P, GC], F32, tag="pb", bufs=2, name="vacc")
                    )
                    for k in range(KC):
                        if v_split:
                            for hh in range(2):
                                nc.tensor.matmul(
                                    vacc[hh * 64 : (hh + 1) * 64, :],
                                    xr[:, k, m * P + hh * 64 : m * P + (hh + 1) * 64],
                                    wvr[:, k, :],
                                    start=(k == 0),
                                    stop=(k == KC - 1),
                                )
                        else:
                            vw = 8 if no_qkv else GC
                            lw = 8 if no_ldw else P
                            packed_mm(
                                vacc[0:lw, 0:vw],
                                xr[:, k, m * P : m * P + lw],
                                wvr[:, k, 0:vw],
                                k == 0,
                                k == KC - 1,
                            )
                    vh = vacc.rearrange("p (h e) -> p h e", h=HPG)
                    nc.vector.tensor_copy(va[:, m, 0::2, 0:HS], vh[:, 0::2, :])
                    nc.vector.tensor_copy(va[:, m, 1::2, HS:P], vh[:, 1::2, :])
                    if not hoist_ones:
                        nc.vector.tensor_copy(va[:, m, 0::2, HS:P], ones_f[:])
                        nc.vector.tensor_copy(va[:, m, 1::2, 0:HS], ones_f[:])

            # ---- phase C: attention for a head pair (2hp, 2hp+1) -----------
            # The two heads' K=64 score matmuls sit at base partitions 0 and
            # 64 -> distinct PE row-groups, so back-to-back emission lets the
            # hardware overlap them.  One exp covers both heads' P tiles.
            def attention_pair(hp, after_q=None):
                for q in range(4):  # query quarters of 512
                    qsl = slice(q * 512, (q + 1) * 512)
                    att0 = psum.tile([P, 512], F32, tag="att0", bufs=1, name="att0")
                    att1 = psum.tile([P, 512], F32, tag="att1", bufs=1, name="att1")
                    for m in range(MC):
                        s = psum.tile([P, 1024], F32, tag="s", bufs=3 if s3 else 2, name="s")
                        sw = 8 if no_scores else 512  # tiny-ablation width
                        lw = 8 if no_ldw else P
                        for par, off in ((0, 0), (1, 64)):
                            o = 0 if no_pair else off
                            nc.tensor.matmul(
                                s[0:lw, par * 512 : par * 512 + sw],
                                kr[o : o + 64, hp, m * P : m * P + lw],
                                qr[o : o + 64, hp, qsl.start : qsl.start + sw],
                                start=True,
                                stop=True,
                            )
                        p_sb = mpool.tile([P, 1024], BF16, tag="p_sb", bufs=8 if slack else 4, name="p_sb")
                        ew = 8 if no_exp else 1024
                        nc.scalar.activation(
                            p_sb[:, 0:ew], s[:, 0:ew], mybir.ActivationFunctionType.Exp, scale=SCALE
                        )
                        aw = 8 if no_av else 512
                        for par, att in ((0, att0), (1, att1)):
                            if av_split:
                                for hh in range(2):
                                    nc.tensor.matmul(
                                        att[hh * 64 : (hh + 1) * 64, :],
                                        va[:, m, 2 * hp + par, hh * 64 : (hh + 1) * 64],
                                        p_sb[:, par * 512 : (par + 1) * 512],
                                        start=(m == 0),
                                        stop=(m == MC - 1),
                                    )
                            else:
                                packed_mm(
                                    att[0 : (8 if no_ldw else P), 0:aw],
                                    va[:, m, 2 * hp + par, 0 : (8 if no_ldw else P)],
                                    p_sb[:, par * 512 : par * 512 + aw],
                                    m == 0,
                                    m == MC - 1,
                                )
                    # normalize.  Even head: attn rows 0:64, rowsum 64:128;
                    # odd head flipped (va block order).  The custom recip
                    # uop only works at base partition 0; cross-partition
                    # moves go through SBUF->SBUF DMA.
                    if norm2:
                        # read att PSUM directly in the muls; odd head's recip
                        # also reads PSUM directly (sums already at rows 0:64).
                        # rr DMAs ride the scalar HWDGE queue, away from outT.
                        au0 = mpool.tile([P, 512], F32, tag="au", bufs=4, name="au0")
                        rr0 = mpool.tile([P, 512], F32, tag="rr", bufs=4, name="rr0")
                        rr1 = mpool.tile([P, 512], F32, tag="rr", bufs=4, name="rr1")
                        nc.vector.tensor_copy(au0[64:128, :], att0[64:128, :])
                        nc.scalar.dma_start(out=rr0[0:64, :], in_=au0[64:128, :])
                        nc.vector.reciprocal_approx_fast(rr1[0:64, :], att1[0:64, :])
                        nc.scalar.dma_start(out=rr1[64:128, :], in_=rr1[0:64, :])
                        nc.vector.reciprocal_approx_fast(rr0[0:64, :], rr0[0:64, :])
                        nc.vector.tensor_mul(an[0:64, hp, qsl], att0[0:64, :], rr0[0:64, :])
                        nc.vector.tensor_mul(
                            an[64:128, hp, qsl], att1[64:128, :], rr1[64:128, :]
                        )
                    else:
                        au0 = mpool.tile([P, 512], F32, tag="au", bufs=6 if slack else 4, name="au0")
                        au1 = mpool.tile([P, 512], F32, tag="au", bufs=6 if slack else 4, name="au1")
                        rr0 = mpool.tile([P, 512], F32, tag="rr", bufs=6 if slack else 4, name="rr0")
                        rr1 = mpool.tile([P, 512], F32, tag="rr", bufs=6 if slack else 4, name="rr1")
                        nc.vector.tensor_copy(au0[:], att0[:])
                        nc.vector.tensor_copy(au1[:], att1[:])
                        (nc.gpsimd if slack else nc.sync).dma_start(out=rr0[0:64, :], in_=au0[64:128, :])
                        nc.vector.reciprocal_approx_fast(rr0[0:64, :], rr0[0:64, :])
                        nc.vector.tensor_mul(an[0:64, hp, qsl], au0[0:64, :], rr0[0:64, :])
                        nc.vector.reciprocal_approx_fast(rr1[0:64, :], au1[0:64, :])
                        (nc.gpsimd if slack else nc.sync).dma_start(out=rr1[64:128, :], in_=rr1[0:64, :])
                        nc.vector.tensor_mul(
                            an[64:128, hp, qsl], au1[64:128, :], rr1[64:128, :]
                        )
                    if after_q is not None:
                        after_q(q)

            # ---- phase E: output projection (one query quarter) ------------
            # 8 och chunks collect into one SBUF tile; a single DMA ships the
            # whole [C, 512] quarter (1 issue instead of 8).
            def out_proj_quarter(nch):
                o_full = opool.tile(
                    [P, C // P, 512], BF16 if out_bf16 else F32, tag="o_full",
                    bufs=2, name="o_full",
                )
                for och in range(C // P):
                    o_ps = (
                        psum.tile([P, 1024], F32, tag="s", bufs=3, name="o_ps")[:, 0:512]
                        if s3
                        else psum.tile([P, 512], F32, tag="pb", bufs=2, name="o_ps")
                    )
                    ow = 8 if no_out else 512
                    lw = 8 if no_ldw else P
                    for c in range(QC):
                        packed_mm(
                            o_ps[0:lw, 0:ow],
                            wor[:, c, och * P : och * P + lw],
                            an[:, c, nch * 512 : nch * 512 + ow],
                            c == 0,
                            c == QC - 1,
                        )
                    nc.vector.tensor_copy(o_full[:, och, :], o_ps[:])
                eng = nc.sync if nch % 2 == 0 else nc.gpsimd
                eng.dma_start(
                    out=outT[:, nch * 512 : (nch + 1) * 512].rearrange(
                        "(o p) n -> p o n", p=P
                    ),
                    in_=o_full[:],
                )

            # ---- body: emission order enables PE/ACT overlap ---------------
            def body(staged):
                if not hoist_ones:
                    nc.vector.memset(ones_f, 1.0)
                if staged:
                    stage_all()
                if v_first:
                    qk_proj(wkr, kr, 0)
                    v_proj()
                    qk_proj(wqr, qr, 0)
                else:
                    qk_proj(wkr, kr, 0)
                    qk_proj(wqr, qr, 0)
                    v_proj()
                attention_pair(0)
                if "qkv2" in parts:
                    qk_proj(wkr, kr, 1)
                    qk_proj(wqr, qr, 1)
                after = out_proj_quarter if ("out" in parts and interleave_out) else None
                if "att2" in parts:
                    attention_pair(1, after_q=after)
                if "out" in parts and after is None:
                    for q in range(4):
                        out_proj_quarter(q)

            if hoist_ones:
                # ones columns of va never change; fill them once up front
                nc.vector.memset(va[:, :, 0::2, HS:P], 1.0)
                nc.vector.memset(va[:, :, 1::2, 0:HS], 1.0)
            if loop_n > 1:
                if not stage_in_loop:
                    stage_all()
                ET = mybir.EngineType
                with tc.For_i(
                    0,
                    loop_n,
                    1,
                    hint_engines=(ET.PE, ET.Activation, ET.DVE, ET.SP),
                ):
                    body(staged=stage_in_loop)
            elif loop_n < 0:  # python-unrolled -loop_n bodies (sim analysis)
                for _ in range(-loop_n):
                    body(staged=True)
            else:
                body(staged=True)

    nc.compile()
    return nc


def shard_inputs(x, w_qkv, w_out):
    """Host-side shard prep. Returns in_maps for cores 0..7 (core = b*4+g).

    All inputs ship as bf16 (the PE consumes bf16 directly at 2x moving
    rate); accumulation on chip is fp32 and the output returns fp32."""
    import ml_dtypes

    bf16 = ml_dtypes.bfloat16
    # w_qkv row d = c_idx*3 + t  (t: 0=q, 1=k, 2=v)  [stride-3 interleave]
    wr = np.ascontiguousarray(w_qkv.reshape(C, 3, C))
    in_maps = []
    for b in range(B):
        xTb = np.ascontiguousarray(x[b].T.astype(bf16))
        for g in range(G):
            sl = slice(g * GC, (g + 1) * GC)
            in_maps.append(
                {
                    "xT": xTb,
                    "wqT": np.ascontiguousarray(wr[sl, 0, :].T.astype(bf16)),
                    "wkT": np.ascontiguousarray(wr[sl, 1, :].T.astype(bf16)),
                    "wvT": np.ascontiguousarray(wr[sl, 2, :].T.astype(bf16)),
                    "woT": np.ascontiguousarray(w_out[:, sl].T.astype(bf16)),
                }
            )
    return in_maps


def kernel(x, w_qkv, w_out, b_out):
    x = np.asarray(x, dtype=np.float32)
    w_qkv = np.asarray(w_qkv, dtype=np.float32)
    w_out = np.asarray(w_out, dtype=np.float32)
    b_out = np.asarray(b_out, dtype=np.float32)

    if "nc" not in _CACHED:
        _CACHED["nc"] = build_bass()
    nc = _CACHED["nc"]

    in_maps = shard_inputs(x, w_qkv, w_out)
    res = run_bass_kernel_spmd(nc, in_maps, core_ids=list(range(8)))

    out = np.empty((B, N, C), dtype=np.float32)
    for b in range(B):
        acc = res.results[b * G + 0]["outT"].astype(np.float32)
        for g in range(1, G):
            acc = acc + res.results[b * G + g]["outT"].astype(np.float32)
        out[b] = acc.T + b_out
    return out


if __name__ == "__main__":
    rng = np.random.default_rng(0)
    x = rng.standard_normal((B, N, C), dtype=np.float32)
    w_qkv = rng.standard_normal((3 * C, C), dtype=np.float32) * C**-0.5
    w_out = rng.standard_normal((C, C), dtype=np.float32) * C**-0.5
    b_out = np.zeros((C,), dtype=np.float32)
    got = kernel(x, w_qkv, w_out, b_out)
    print("kernel ran, output shape", got.shape)

